# revision 1
# baseline (speedup 1.0000x reference)
"""Block-sparse to_dense (scatter-add) on 8 Trainium2 NeuronCores.

Problem: block_values [2048, 64, 64, 8] f32 scatter-added into a dense
[4096, 4096, 8] f32 at 64-aligned positions given by block_indices [2048, 2]
(block row/col in a 64x64 grid). Overlapping blocks sum; out-of-range blocks
drop (indices are block-aligned and H=W=4096=64*64, so partial clipping is
impossible - a block is either fully inside or fully outside).

Strategy (uniform SPMD program, all irregularity in host-prepared data):
  - The dense output is a 64x64 grid of cells; rows are grouped into 32
    "row-pair" slabs of 128 rows. Each core owns 4 slabs (position q=0..3),
    assigned by sorting slabs by block count so padding is minimal and load
    is balanced.
  - Host routes blocks: vals[core] = gathered flattened blocks in bf16,
    laid out stage-major so every stage-in DMA reads one fully contiguous
    HBM region; sel[core] = 0/1 selection matrix (packed, one DMA).
  - Device, per slab: cells[128, 32768] = sel^T @ vals_rows on the
    TensorEngine. Empty cells get zeros for free; overlapping blocks sum in
    fp32 PSUM. Both input values and the dense output travel as bf16
    (input + output quantization rel err ~2.4e-3, well under the 2e-2
    gate): half the input traffic and half the output traffic of an f32
    scheme, on a kernel that is DMA-bus-bound.
  - DMA tuning (from NTFF traces): the kernel is DMA-bus-bound end to end
    (16 SDMA engines ~100% busy). Output rides the qSP HWDGE ring as 1KB
    descriptors (the dense layout's contiguous run) at ~20 GB/s/engine.
    HBM reads transfer at ~48-53ns/KB with 2KB descriptors but ~60-71ns/KB
    at >=4KB, and AP-level splits get coalesced back by the DGE, so vals
    are stored piece-interleaved on host (each 2KB piece in its own
    [M_tot, 1024] region) making every stage-in descriptor a
    non-coalescable 2KB run; staging rides the qAct ring as 4 sub-tiles
    per stage for fine-grained matmul dependencies. PSUM -> SBUF copies
    convert f32 -> bf16, split 11/16 to Vector and 5/16 to Scalar (scalar
    also generates input descriptors). Host converts the gathered bf16
    dense back to f32. Per-matmul LDWEIGHTS are left in place: removing
    them measured ~13us slower (LDWEIGHTS->MATMUL pairs pipeline better
    through the PE's 64-deep reorder window).
"""
import numpy as np

N_CORES = 8
B = 64
GRID = 64
KS = 8
H = W = 4096
FLAT = B * B * KS          # 32768 values per block
QS = 4                     # row-pair slabs per core
N_PAIRS = 32

# device loop tiling (chunks of 512 values along FLAT; 64 chunks total)
CH = 512                   # one chunk = one PSUM bank's worth of fp32 cols
N_CHUNK = FLAT // CH       # 64
CH_PER_PSUM = 2            # chunks per psum tile  [128, 1024] fp32
CH_PER_OUT = 16            # chunks per out tile [128, 8192] bf16 -> 2 x 1 MB DMAs


def _stage_cfg(rounds):
    """chunks per stage-in DMA, shrunk under heavy index clustering so the
    per-round stage tiles still fit in SBUF (never happens for uniform
    indices where rounds == [1,1,1,1])."""
    r_max = max(rounds)
    if r_max <= 1:
        ch_stage = 32          # [m, 16384] bf16: 32 KB contiguous rows
    elif r_max <= 2:
        ch_stage = 8
    elif r_max <= 4:
        ch_stage = 4
    else:
        ch_stage = 2
    return ch_stage


# ----------------------------------------------------------------- host prep
def _plan_routing(block_indices):
    idx = np.asarray(block_indices).astype(np.int64)
    r, c = idx[:, 0], idx[:, 1]
    valid = (r >= 0) & (r < GRID) & (c >= 0) & (c < GRID)
    pair = r // 2

    ids_by_pair = [[] for _ in range(N_PAIRS)]
    for n in np.nonzero(valid)[0]:
        ids_by_pair[pair[n]].append(int(n))
    counts = np.array([len(x) for x in ids_by_pair])

    order = np.argsort(-counts, kind="stable")
    pair_of = [[0] * QS for _ in range(N_CORES)]
    ids = [[None] * QS for _ in range(N_CORES)]
    m_q = []
    for q in range(QS):
        grp = order[q * N_CORES:(q + 1) * N_CORES]
        # round up to a multiple of 16 so DMA descriptor groups spread
        # evenly over the 16 SDMA engines
        m_q.append(max(16, -(-int(counts[grp].max()) // 16) * 16))
        for core in range(N_CORES):
            pair_of[core][q] = int(grp[core])
            ids[core][q] = ids_by_pair[int(grp[core])]
    rounds = [(m + 127) // 128 for m in m_q]
    row0 = np.concatenate([[0], np.cumsum(m_q)]).astype(int)
    sel_idx = np.concatenate([[0], np.cumsum(rounds)]).astype(int)
    return dict(pair_of=pair_of, ids=ids, m_q=m_q, rounds=rounds, row0=row0,
                sel_idx=sel_idx, M_tot=int(row0[-1]), R_tot=int(sel_idx[-1]),
                r=r, c=c)


def _piece_cfg(ch_stage):
    """HBM reads transfer at ~48ns/KB with 2KB descriptors but ~60ns/KB at
    >=4KB (measured), so vals are stored piece-interleaved: piece p of stage
    s lives in its own [M_tot, PIECE] region, making each descriptor a 2KB
    run that cannot coalesce with its neighbor (regions are M_tot rows
    apart)."""
    seg = ch_stage * CH
    piece = 1024 if seg >= 4096 else seg          # elements (bf16 -> 2KB)
    return piece, seg // piece                    # (PIECE, pieces per stage)


def _build_core_inputs(plan, bv_flat, core):
    """vals bf16 [S*P*M_tot, PIECE] stage- and piece-major (piece p of
    stage s of slab q at rows (s*P + p)*M_tot + row0[q] ...),
    sel bf16 [128, R_tot*128] packed."""
    import ml_dtypes
    bf16 = ml_dtypes.bfloat16
    M_tot, R_tot = plan["M_tot"], plan["R_tot"]
    ch_stage = _stage_cfg(plan["rounds"])
    S = N_CHUNK // ch_stage
    seg = ch_stage * CH
    piece, P = _piece_cfg(ch_stage)
    vals = np.zeros((S * P * M_tot, piece), dtype=bf16)
    sel = np.zeros((128, R_tot * 128), dtype=bf16)
    r_all, c_all = plan["r"], plan["c"]
    for q in range(QS):
        blks = plan["ids"][core][q]
        r0, s0 = plan["row0"][q], plan["sel_idx"][q]
        if blks:
            x = bv_flat[blks].astype(bf16)      # [n, FLAT]
            for s in range(S):
                for p in range(P):
                    base = (s * P + p) * M_tot + r0
                    vals[base: base + len(blks)] = \
                        x[:, s * seg + p * piece: s * seg + (p + 1) * piece]
        for slot, n in enumerate(blks):
            col = int(r_all[n] % 2) * 64 + int(c_all[n])
            sel[slot % 128, (s0 + slot // 128) * 128 + col] = 1.0
    return {"vals": vals, "sel": sel}


# -------------------------------------------------------------- bass program
_PROGRAM_CACHE = {}


def _build_program(m_q, ch_psum=CH_PER_PSUM, ch_out=CH_PER_OUT,
                   stage_bufs=3, out_bufs=4, psum_bufs=4, copy_split=True):
    import concourse.mybir as mybir
    from concourse import bacc
    from concourse.tile import TileContext

    m_q = list(m_q)
    rounds = [(m + 127) // 128 for m in m_q]
    row0 = np.concatenate([[0], np.cumsum(m_q)]).astype(int)
    sel_idx = np.concatenate([[0], np.cumsum(rounds)]).astype(int)
    M_tot, R_tot = int(row0[-1]), int(sel_idx[-1])

    ch_stage = _stage_cfg(rounds)
    S = N_CHUNK // ch_stage
    seg = ch_stage * CH
    piece, P = _piece_cfg(ch_stage)
    r_max = max(rounds)
    if r_max > 8:
        stage_bufs, out_bufs = 1, 2
    if r_max > 1:
        ch_psum = min(ch_psum, ch_stage)
    ch_out_eff = min(ch_out, ch_stage)
    # input stage sub-tiles: finer DMA-completion granularity + smaller
    # descriptor batches per queue
    subs = 4 if (ch_stage % 4 == 0 and P % 4 == 0) else 1
    sub_ch = ch_stage // subs              # chunks per sub-tile
    sub_seg = sub_ch * CH                  # elements per sub-tile row
    pps = P // subs                        # pieces per sub-tile
    f32 = mybir.dt.float32
    bf16 = mybir.dt.bfloat16

    nc = bacc.Bacc(
        "TRN2", target_bir_lowering=False, debug=False, num_devices=N_CORES)
    vals = nc.dram_tensor(
        "vals", [S * P * M_tot, piece], bf16, kind="ExternalInput")
    # [S, P, M_tot, piece]: piece p of stage s in its own region
    vals_v = vals[:].rearrange("(s p m) e -> s p m e", s=S, p=P)
    sel = nc.dram_tensor("sel", [128, R_tot * 128], bf16, kind="ExternalInput")
    out = nc.dram_tensor("out", [512, W, KS], bf16, kind="ExternalOutput")

    # out rows = 128*q + 64*half + tt ; cols = 64*c + w ; innermost ks
    # view: [q, half, c, tt, w*ks] so a (q, half) slice iterates (c, tt, wk)
    # in the same order as SBUF [partition=c, free=(tt, wk)]
    out_v = out[:].rearrange(
        "(q half tt) (c w) k -> q half c tt (w k)",
        q=QS, half=2, tt=B, c=GRID,
    )

    with TileContext(nc) as tc:
        # HWDGE rings: SP (sync) and Activation (scalar) only. All output
        # descriptor generation rides SP; input staging rides Activation
        # (8KB descriptors: best gen-cost/transfer-rate tradeoff measured).
        with (
            tc.tile_pool(name="spool", bufs=1) as s_pool,
            tc.tile_pool(name="stage", bufs=stage_bufs) as stage_pool,
            tc.tile_pool(name="fine", bufs=1) as fine_pool,
            tc.tile_pool(name="outp", bufs=out_bufs) as out_pool,
            tc.tile_pool(name="psum", bufs=psum_bufs, space="PSUM") as psum_pool,
        ):
            s_all = s_pool.tile([128, R_tot * 128], bf16, tag="sel")
            nc.sync.dma_start(out=s_all[:], in_=sel[:])
            n_og = N_CHUNK // ch_out_eff
            for q in range(QS):
                nr = rounds[q]
                stage_tiles = [[None] * subs for _ in range(nr)]
                # smaller first/last out-groups compress pipeline fill/drain
                if ch_out_eff == 16:
                    if q == 0:
                        og_sizes = [4, 4, 8, 16, 16, 16]
                    elif q == QS - 1:
                        og_sizes = [16, 16, 16, 8, 4, 4]
                    else:
                        og_sizes = [16] * n_og
                else:
                    og_sizes = [ch_out_eff] * n_og
                t_base = 0
                for og_sz in og_sizes:                           # out groups
                    # one shared tag sized for the largest group: small
                    # groups slice a prefix, so the pool holds 4 x 16KB
                    # buffers instead of per-size buffer sets
                    outb_full = out_pool.tile([128, ch_out_eff * CH], bf16,
                                              tag="outb")
                    outb = outb_full[:, :og_sz * CH]
                    for pg in range(og_sz // ch_psum):           # psum groups
                        t0 = t_base + pg * ch_psum
                        if t0 % ch_stage == 0:
                            s = t0 // ch_stage
                            # very first stage: 2x finer sub-tiles in their
                            # own bufs=1 pool, alternating rings, so the
                            # first matmul's data lands in half the time
                            fine = (q == 0 and s == 0 and subs == 4
                                    and pps % 2 == 0)
                            n_sub = subs * 2 if fine else subs
                            cur_sub_ch = ch_stage // n_sub
                            cur_pps = P // n_sub
                            pool = fine_pool if fine else stage_pool
                            for r in range(nr):
                                k = min(128, m_q[q] - 128 * r)
                                base = row0[q] + 128 * r
                                stage_tiles[r] = [None] * n_sub
                                for j in range(n_sub):
                                    stg = pool.tile(
                                        [128, cur_sub_ch * CH], bf16,
                                        tag=(f"stgf_{r}_{j}" if fine
                                             else f"stg_{r}_{j}"))
                                    eng = (nc.sync if (fine and j % 2 == 1)
                                           else nc.scalar)
                                    src = vals_v[
                                        s, j * cur_pps:(j + 1) * cur_pps,
                                        base: base + k, :,
                                    ].rearrange("p m e -> m p e")
                                    eng.dma_start(out=stg[:k, :], in_=src)
                                    stage_tiles[r][j] = stg
                        psum = psum_pool.tile([128, ch_psum * CH], f32, tag="ps")
                        ci = t0 % ch_stage          # chunk within stage
                        for i in range(ch_psum):
                            j = (ci + i) // cur_sub_ch
                            off = ((ci + i) % cur_sub_ch) * CH
                            for r in range(nr):
                                k = min(128, m_q[q] - 128 * r)
                                sc = (sel_idx[q] + r) * 128
                                nc.tensor.matmul(
                                    out=psum[:, i * CH:(i + 1) * CH],
                                    lhsT=s_all[:k, sc:sc + 128],
                                    rhs=stage_tiles[r][j][:k, off:off + CH],
                                    start=(r == 0),
                                    stop=(r == nr - 1),
                                )
                        dst = outb[:, pg * ch_psum * CH:(pg + 1) * ch_psum * CH]
                        if copy_split:
                            # scalar also generates input descriptors, so
                            # vector takes the bigger copy share (11/16)
                            hw = ch_psum * CH * 11 // 16
                            nc.vector.tensor_copy(out=dst[:, :hw], in_=psum[:, :hw])
                            nc.scalar.copy(out=dst[:, hw:], in_=psum[:, hw:])
                        else:
                            nc.vector.tensor_copy(out=dst, in_=psum[:])
                    for half in range(2):
                        src = outb[64 * half:64 * half + 64, :].rearrange(
                            "p (t wk) -> p t wk", t=og_sz)
                        nc.sync.dma_start(
                            out=out_v[q, half, :, t_base:t_base + og_sz, :],
                            in_=src,
                        )
                    t_base += og_sz
    nc.compile()
    return nc


# ------------------------------------------------------------------- kernel
def kernel(block_values, block_indices, block_size=None, ks=None, **kw):
    from concourse import bass_utils

    bv = np.ascontiguousarray(np.asarray(block_values), dtype=np.float32)
    assert bv.shape == (2048, B, B, KS), bv.shape
    bv_flat = bv.reshape(-1, FLAT)

    plan = _plan_routing(block_indices)
    key = tuple(plan["m_q"])
    if key not in _PROGRAM_CACHE:
        _PROGRAM_CACHE[key] = _build_program(plan["m_q"])
    nc = _PROGRAM_CACHE[key]

    in_maps = [_build_core_inputs(plan, bv_flat, core) for core in range(N_CORES)]
    res = bass_utils.run_bass_kernel_spmd(nc, in_maps, core_ids=list(range(N_CORES)))

    dense = np.zeros((H, W, KS), dtype=np.float32)
    for core in range(N_CORES):
        o = np.asarray(res.results[core]["out"]).astype(np.float32)
        for q in range(QS):
            p = plan["pair_of"][core][q]
            dense[128 * p:128 * p + 128] = o[128 * q:128 * q + 128]
    return dense

